# revision 46
# baseline (speedup 1.0000x reference)
"""Trainium2 Bass kernel for nn_ChimeraNet (encoder -> 10-step Euler RNN -> LN -> readout).

Data-parallel over 8 NeuronCores: each core gets 1024 rows of the batch and a
replicated set of (host-prefolded) weights.

Math (per core, R=1024 rows, D=1024), in "drive space" z = h @ W_res + c:
    c   = x @ W_c + bias               with W_c = W_enc.T @ W_in (host-folded)
    z_0 = c;  T_k = tanh(z_k)
    z_{k+1} = 0.8 z_k + 0.2 c + 0.2 (T_k @ W_res)      k = 0..8
    u_{k+1} = 0.8 u_k + T_k                            k = 0..9,  u_0 = T_0
    h = 0.2 u_10;  out = LayerNorm(h) @ W_out.T + b_out (folded)

The z state is kept in the exponentially rescaled+upscaled frame
G_k = 16 z_k / 0.8^k (bf16) so each step's state update is a single
one-scalar DVE op reading the matmul PSUM directly:
    G_{k+1} = G_k + 1.25^{k+1} * psum
    psum    = 16 c (bf16 identity matmul) + T8 @ fp8(16 W_res)  (DoubleRow fp8)
    T_k     = tanh((0.8^k/16) * G_k)   (ACT with scale, fp8 out - no cast op)
The drive tiles store 16c in bf16 (the host folds the 16 into W_c), so the
identity matmuls run at bf16 rate and G_0 IS the drive tile.

Work distribution per Euler step (per-core, per [128,1024] tile x8):
    PE   : bf16 identity (re-add 16c) + 4x fp8-DR matmuls       ~17.4 us
    DVE  : 8 G-updates (STT from PSUM) + 5 u-updates (STT)      ~16 us
    ACT  : 8 fp8 tanhs + 5 exact bf16 tanhs for the DVE u's     ~14.7 us
    Pool : 3 u pre-decays + 3 u adds (m5-7, reading fp8 tanh)   ~14 us
Each step runs as 5 phases (m0 | m1 | m2,m3 | m4,m5 | m6,m7) over a 4-deep
[128,1024] psum rotation; splitting the first pair makes G0 (and with it the
13-op ACT chain the next step's matmuls wait on) start ~4us earlier.  The T
tiles are 4 per-k-pair fp8 tiles (own semaphores: a matmul waits only on the
2 tanhs it reads), double-buffered by step parity, and each phase's j-group
order is rotated to match when the previous step's tanhs land.  The last
step is Pool-free, processes tiles in reverse, and the readout matmuls sweep
k in that completion order; LN + readout run as batched [128,8]-wide ops on
a stacked transpose target with stride-0 broadcasts.

fp8 e4m3 is used for the recurrent matmul operands (T8 = direct fp8 tanh;
W8 = fp8(16 W_res)) and for the Pool-side u accumulation; the final tanh
T_9 (u-weight 1) and the 5 DVE-side u tiles use exact bf16 tanh, keeping
the final relative error ~1.1e-2.
"""

import os
import sys

import numpy as np

try:
    import concourse.bass as bass  # noqa: F401
except ImportError:  # pragma: no cover - fresh grading env without PYTHONPATH
    for p in ("/root/.axon_site", "/root/.axon_site/_ro/trn_rl_repo",
              "/root/.axon_site/_ro/pypackages", "/opt/trn_rl_repo"):
        if os.path.isdir(p) and p not in sys.path:
            sys.path.append(p)
    import concourse.bass as bass

from contextlib import ExitStack

import ml_dtypes
import concourse.tile as tile
from concourse import bacc, bass_utils, mybir
from concourse.masks import make_identity

N_CORES = 8
B = 8192
R = B // N_CORES        # rows per core
D = 1024                # latent dim
KX = 784                # encoder input dim
KE = 7                  # padded encoder k tiles (896 = 7*128)
DT_STEP = 0.2
STEPS = 10
EPS = 1e-5
SW = 16.0               # fp8 weight upscale (exact in bf16/f32)

F32 = mybir.dt.float32
BF16 = mybir.dt.bfloat16
F8 = mybir.dt.float8e4
AF = mybir.ActivationFunctionType
ALU = mybir.AluOpType
DR = mybir.MatmulPerfMode.DoubleRow

KD = D // 128           # 8 k/m tiles over D
NS = R // 512           # 2 moving-dim slices of 512 (psum bank width)
NWARM = 16              # PE warmup matmuls (hold clock while DMAs land)


def _build_program():
    nc = bacc.Bacc("TRN2", target_bir_lowering=False, debug=False)

    xt = nc.dram_tensor("xt", [128, KE, R], BF16, kind="ExternalInput").ap()
    wc = nc.dram_tensor("wc", [128, KE, D], BF16, kind="ExternalInput").ap()
    w8 = nc.dram_tensor("w8", [128, KD, D], F8, kind="ExternalInput").ap()
    bias = nc.dram_tensor("bias", [128, KD], F32, kind="ExternalInput").ap()
    w2a = nc.dram_tensor("w2a", [128, KD, 11], BF16, kind="ExternalInput").ap()
    w2r = nc.dram_tensor("w2r", [128, KD, 11], BF16, kind="ExternalInput").ap()
    w1 = nc.dram_tensor("w1", [10], F32, kind="ExternalInput").ap()
    b2 = nc.dram_tensor("b2", [10], F32, kind="ExternalInput").ap()
    out = nc.dram_tensor("out", [R, 10], F32, kind="ExternalOutput").ap()

    with tile.TileContext(nc) as tc, ExitStack() as ctx:
        state = ctx.enter_context(tc.tile_pool(name="state", bufs=1))
        consts = ctx.enter_context(tc.tile_pool(name="consts", bufs=1))
        wres_pool = ctx.enter_context(tc.tile_pool(name="wres", bufs=1))

        # persistent SBUF state (G in fp32 updated in place, u in bf16,
        # drive holds 16c in bf16)
        g = [state.tile([128, R], BF16, name=f"g{k}", tag=f"g{k}") for k in range(KD)]
        u = [state.tile([128, R], BF16, name=f"u{k}", tag=f"u{k}") for k in range(KD)]
        drive = [state.tile([128, R], BF16, name=f"dr{k}", tag=f"dr{k}")
                 for k in range(KD)]
        # T in fp8, split into per-k-pair tiles (own semaphores -> matmuls
        # wait only on the 2 casts they read) and double-buffered by step
        # parity (no WAR between step s's casts and step s's matmuls).
        t8p = [[state.tile([128, 2, R], F8, name=f"t8_{b}_{jj}", tag=f"t8_{b}_{jj}")
                for jj in range(4)] for b in range(2)]
        w8_sb = wres_pool.tile([128, KD, D], F8, name="w8", tag="w8")

        with ExitStack() as mmctx:
            # one psum pool: 4 x [128,1024] f32 = all 8 banks
            psum = mmctx.enter_context(
                tc.tile_pool(name="mm", bufs=4, space="PSUM"))

            # input DMAs first (queues fill while PE warms up)
            xt_pool = ctx.enter_context(tc.tile_pool(name="xt", bufs=1))
            wc_pool = ctx.enter_context(tc.tile_pool(name="wc", bufs=1))
            xt_sb = xt_pool.tile([128, KE, R], BF16, name="xt")
            wc_sb = wc_pool.tile([128, KE, D], BF16, name="wc")
            # encoder-critical 3.5MB (xt + wc) balanced ~1.17MB across all
            # three DGE trigger queues; w8 follows on gpsimd (needed only
            # when the Euler loop starts).
            nc.sync.dma_start(out=xt_sb[:, :4, :], in_=xt[:, :4, :])
            nc.gpsimd.dma_start(out=xt_sb[:, 4:, :], in_=xt[:, 4:, :])
            nc.gpsimd.dma_start(out=wc_sb[:, 5:, :], in_=wc[:, 5:, :])
            nc.scalar.dma_start(out=wc_sb[:, :5, :], in_=wc[:, :5, :])
            bias_sb = consts.tile([128, KD], F32)
            nc.gpsimd.dma_start(out=bias_sb, in_=bias)
            nc.gpsimd.dma_start(out=w8_sb, in_=w8)

            # tail weights (tiny, same cheap gpsimd queue)
            tail = ctx.enter_context(tc.tile_pool(name="tail", bufs=1))
            w2a_sb = tail.tile([128, KD, 11], BF16)
            nc.gpsimd.dma_start(out=w2a_sb, in_=w2a)
            w2r_sb = tail.tile([128, KD, 11], BF16)
            nc.gpsimd.dma_start(out=w2r_sb, in_=w2r)
            w1_bc = tail.tile([128, 10], F32)
            nc.gpsimd.dma_start(out=w1_bc, in_=bass.AP(tensor=w1.tensor, offset=w1.offset,
                                                       ap=[[0, 128]] + list(w1.ap)))
            b2_bc = tail.tile([128, 10], F32)
            nc.gpsimd.dma_start(out=b2_bc, in_=bass.AP(tensor=b2.tensor, offset=b2.offset,
                                                       ap=[[0, 128]] + list(b2.ap)))

            # PE warmup: dependency-free f32 matmuls pull the clock gate to
            # full speed while the input DMAs are in flight.
            warm_src = consts.tile([128, 512], F32)
            nc.vector.memset(warm_src, 0.01)
            warm_sb = consts.tile([128, 1], F32)
            for w in range(NWARM):
                wp = psum.tile([128, 512], F32, name=f"warm{w}", tag="mm")
                nc.tensor.matmul(wp, lhsT=warm_src[:, :128], rhs=warm_src,
                                 start=True, stop=True)
                if w == NWARM - 1:
                    nc.vector.tensor_copy(warm_sb, wp[:, :1])  # keep-alive

            ident = consts.tile([128, 128], F32)
            make_identity(nc, ident)
            ident16 = consts.tile([128, 128], BF16)
            nc.vector.tensor_copy(ident16, ident)
            # broadcast 0.8 tile: lets the Pool engine do u *= 0.8 as a plain
            # tensor_tensor (Pool supports neither STT nor tensor_scalar)
            decay_sb = consts.tile([128, R], BF16)
            nc.vector.memset(decay_sb, 1.0 - DT_STEP)

            tau_pool = ctx.enter_context(tc.tile_pool(name="tau", bufs=6))

            # ------------ encoder: 16c = xT.T @ (16 W_c) + 16 bias ----------
            # m-outer so each m's eviction+prologue hides under the next m's
            # matmul sweep.
            for m in range(KD):
                ps = psum.tile([128, R], F32, name=f"eps{m}", tag="mm")
                for n in range(NS):
                    sl = slice(n * 512, (n + 1) * 512)
                    for k in range(KE):
                        nc.tensor.matmul(
                            ps[:, sl],
                            lhsT=wc_sb[:, k, m * 128:(m + 1) * 128],
                            rhs=xt_sb[:, k, sl],
                            start=(k == 0), stop=(k == KE - 1))
                # evict 16c + 16bias -> bf16 drive (G_0 frame)
                nc.scalar.activation(drive[m], ps, AF.Identity,
                                     bias=bias_sb[:, m:m + 1], scale=1.0)
                # prologue: T_0 = tanh(z_0) straight to fp8 (G_0 = 16c);
                # u_0 = T_0 via fp8->bf16 copy
                nc.scalar.activation(t8p[0][m // 2][:, m % 2, :], drive[m],
                                     AF.Tanh, scale=float(1.0 / SW))
                nc.vector.tensor_copy(u[m], t8p[0][m // 2][:, m % 2, :])

            sqp = ctx.enter_context(tc.tile_pool(name="sq", bufs=1))
            sq_tiles = [sqp.tile([128, R], BF16, name=f"sq{k}", tag=f"sq{k}")
                        for k in range(KD)]

            # ------------ Euler integration loop (16z/0.8^k frame) ----------
            def mm_id(ps, m):
                for n in range(NS):
                    sl = slice(n * 512, (n + 1) * 512)
                    nc.tensor.matmul(ps[:, sl], lhsT=ident16,
                                     rhs=drive[m][:, sl],
                                     start=True, stop=False)

            def mm_f8(ps, m, j, stop, rbuf):
                lhsT = w8_sb[:, 2 * j:2 * j + 2, m * 128:(m + 1) * 128]
                for n in range(NS):
                    sl = slice(n * 512, (n + 1) * 512)
                    nc.tensor.matmul(ps[:, sl], lhsT=lhsT,
                                     rhs=t8p[rbuf][j][:, :, sl],
                                     perf_mode=DR,
                                     start=False, stop=stop)

            # Per step: 4 pair-phases (2 m-tiles each, psum 4-buf rotation).
            # Pair p runs its fp8 j-groups in rotated order ending with
            # k-pair p, so no matmul ever waits on the previous step's late
            # tanh/casts (k6/k7 feed pair2's FIRST group, issued ~9us in).
            # DVE interleaves u-updates into its psum-wait bubbles.
            def u_upd(s, m, tau, last):
                # u_{s+2} = 0.8 u_{s+1} + T_{s+1}; m0-4 on DVE (STT), m5-7 on
                # Pool (pre-decayed tensor add). tau is the fp8 tanh slice
                # except on the last step (exact bf16 -- T_9 has u-weight 1).
                # Last step avoids Pool entirely: its slow, late-draining
                # queue would otherwise pace the readout.
                if m < 5 or last:
                    nc.vector.scalar_tensor_tensor(
                        u[m], in0=u[m], scalar=1.0 - DT_STEP,
                        in1=tau, op0=ALU.mult, op1=ALU.add)
                else:
                    nc.gpsimd.tensor_add(u[m], u[m], tau)
                if last:
                    nc.vector.tensor_mul(sq_tiles[m], u[m], u[m])

            for s in range(STEPS - 1):
                ak1 = float(0.8 ** (s + 1) / SW)       # tanh scale, step s+1
                qk = float(DT_STEP * 1.25 ** (s + 1))  # G-update scalar
                cur = drive if s == 0 else g
                last = (s + 1 == STEPS - 1)
                rbuf, wbuf = s % 2, (s + 1) % 2
                taus = [None] * KD
                # Normal steps split the first pair into single-m phases so
                # G0 (and with it the whole ACT tanh chain) starts ~4us
                # earlier -- the next step's matmuls wait on that chain.
                # The last step instead processes Pool's tiles (m5-7) first
                # so the tail's readout pacing starts on finished tiles.
                # per-phase fp8 j-group order, matched to when the
                # previous step's tanhs land (t8p[3] is the latest)
                jorders = [[1, 2, 0, 3], [2, 0, 1, 3],
                           [0, 1, 2, 3], [1, 2, 3, 0], [2, 3, 0, 1],
                           [0, 1, 2, 3]]
                if last:
                    # singles at both ends: m6 early starts the readout
                    # pacing; m0/m1 singles land the final u/sq sooner
                    phases = [(6,), (7,), (4, 5), (0,), (1,), (2, 3)]
                else:
                    phases = [(0,), (1,), (2, 3), (4, 5), (6, 7)]
                prev_ms = None
                # Pool pre-decay for its u tiles (no deps on this step's taus)
                if not last:
                    for m in range(5, KD):
                        nc.gpsimd.tensor_mul(u[m], u[m], decay_sb)
                for q, ms in enumerate(phases):
                    pss = {}
                    for m in ms:
                        pss[m] = psum.tile([128, R], F32, name=f"ps{s}_{m}",
                                           tag="mm")
                        mm_id(pss[m], m)
                    jorder = jorders[q]
                    for jpos, j in enumerate(jorder):
                        for m in ms:
                            mm_f8(pss[m], m, j, stop=(jpos == 3), rbuf=rbuf)
                    for m in ms:
                        nc.vector.scalar_tensor_tensor(
                            g[m], in0=pss[m], scalar=qk,
                            in1=cur[m], op0=ALU.mult, op1=ALU.add)
                    # fp8 tanhs first (they feed the next step's matmuls);
                    # the bf16 double-tanhs for the DVE u-STTs have slack
                    for m in ms:
                        if last:
                            # final tanh in exact bf16 for the u accumulator
                            tau = tau_pool.tile([128, R], BF16,
                                                name=f"tau9_{m}", tag="tau")
                            nc.scalar.activation(tau, g[m], AF.Tanh, scale=ak1)
                            taus[m] = tau
                        else:
                            # tanh straight to fp8 (no separate cast op)
                            dst = t8p[wbuf][m // 2][:, m % 2, :]
                            nc.scalar.activation(dst, g[m], AF.Tanh, scale=ak1)
                            taus[m] = dst
                    if not last:
                        # second, exact bf16 tanhs for the DVE u-STTs (fp8
                        # in1 runs ~2.5x slower on DVE; ACT has slack and u
                        # gets full tanh precision). b3/b4 are deferred
                        # behind f6/f7 so the last matmul-gating fp8 tanhs
                        # land ~2us earlier in ACT's queue; their consumers
                        # (u3/u4) run at the end of the DVE queue.
                        bms = [m for m in ms if m < 3]
                        if q == len(phases) - 1:
                            bms.extend([3, 4])
                        for m in bms:
                            tau = tau_pool.tile([128, R], BF16,
                                                name=f"tau{s + 1}_{m}",
                                                tag="tau")
                            nc.scalar.activation(tau, g[m], AF.Tanh,
                                                 scale=ak1)
                            taus[m] = tau
                    # u-updates for the PREVIOUS phase slot into DVE bubbles
                    # (u3 deferred to the end alongside its late tanh)
                    if prev_ms is not None:
                        for m in prev_ms:
                            if last or m != 3:
                                u_upd(s, m, taus[m], last)
                    prev_ms = ms
                for m in prev_ms:
                    u_upd(s, m, taus[m], last)
                if not last:
                    u_upd(s, 3, taus[3], last)

            # ------------ tail: LN stats + readout (matmul part) ------------
            ones_sb = tail.tile([128, 1], BF16)
            nc.vector.memset(ones_sb, 1.0)
            eps_sb = tail.tile([128, 1], F32)
            nc.vector.memset(eps_sb, EPS)

            s2_sb = tail.tile([1, R], F32)
            y_sb = tail.tile([11, R], F32)

            # y matmuls first (paced by u finalization: the last Euler step
            # processes pairs in reverse, so sweep k in that completion
            # order), s2 after (paced by the sq tiles).
            KORD = [6, 7, 4, 5, 0, 1, 2, 3]
            yps = [psum.tile([11, 512], F32, name=f"yp{n}", tag="mm")
                   for n in range(NS)]
            for ki, k in enumerate(KORD):
                for n in range(NS):
                    sl = slice(n * 512, (n + 1) * 512)
                    nc.tensor.matmul(yps[n], lhsT=w2a_sb[:, k, :],
                                     rhs=u[k][:, sl],
                                     start=(ki == 0), stop=False)
            s2s = [psum.tile([1, 512], F32, name=f"s2p{n}", tag="mm")
                   for n in range(NS)]
            for ki, k in enumerate(KORD):
                for n in range(NS):
                    sl = slice(n * 512, (n + 1) * 512)
                    nc.tensor.matmul(s2s[n], lhsT=ones_sb,
                                     rhs=sq_tiles[k][:, sl],
                                     start=(ki == 0), stop=(ki == KD - 1))
            for n in range(NS):
                nc.scalar.copy(s2_sb[:, n * 512:(n + 1) * 512], s2s[n])
            for ki, k in enumerate(KORD):
                for n in range(NS):
                    sl = slice(n * 512, (n + 1) * 512)
                    nc.tensor.matmul(yps[n], lhsT=w2r_sb[:, k, :],
                                     rhs=u[k][:, sl],
                                     start=False, stop=(ki == KD - 1))
            for n in range(NS):
                nc.scalar.copy(y_sb[:, n * 512:(n + 1) * 512], yps[n])

            mmctx.close()

            def bc(ap, n, axis):
                # broadcast an AP along a new stride-0 dim inserted at `axis`
                newap = list(ap.ap)
                newap.insert(axis, [0, n])
                return bass.AP(tensor=ap.tensor, offset=ap.offset, ap=newap)

            # batched LN + readout: transpose all 8 row-tiles into one stacked
            # [128, rt, 12] psum tile, then do the whole LN/readout chain as
            # [128,8]-wide ops instead of 8 serial per-rt chains.
            tp2ctx = ExitStack()
            tp2 = tp2ctx.enter_context(
                tc.tile_pool(name="tp2", bufs=1, space="PSUM"))
            tp_all = tp2.tile([128, 8, 12], F32, name="tp_all")
            for rt in range(8):
                sl = slice(rt * 128, (rt + 1) * 128)
                nc.tensor.transpose(tp_all[:, rt, 0:11], y_sb[:, sl],
                                    ident[:11, :11])
                nc.tensor.transpose(tp_all[:, rt, 11:12], s2_sb[:, sl],
                                    ident[:1, :1])
            st_all = tail.tile([128, 8, 12], F32, name="st_all")
            nc.vector.tensor_copy(st_all, tp_all)
            yn_all = st_all[:, :, 0:10]
            mu_n = tail.tile([128, 8, 1], F32, name="mu_all")
            nc.scalar.mul(mu_n, st_all[:, :, 10:11], -DT_STEP / D)  # -mean(h)
            ex2 = tail.tile([128, 8, 1], F32, name="ex2_all")
            nc.scalar.mul(ex2, st_all[:, :, 11:12], DT_STEP * DT_STEP / D)
            var = tail.tile([128, 8, 1], F32, name="var_all")
            nc.vector.scalar_tensor_tensor(var, in0=mu_n, scalar=-1.0,
                                           op0=ALU.mult, in1=mu_n,
                                           op1=ALU.mult)      # -mean^2
            nc.vector.tensor_add(var, var, ex2)
            sd = tail.tile([128, 8, 1], F32, name="sd_all")
            nc.scalar.activation(sd, var, AF.Sqrt, bias=eps_sb, scale=1.0)
            inv = tail.tile([128, 8, 1], F32, name="inv_all")
            nc.vector.reciprocal(inv, sd)
            qn = tail.tile([128, 8, 1], F32, name="qn_all")
            nc.vector.tensor_mul(qn, mu_n, inv)                     # -mu*inv

            o_all = tail.tile([128, 8, 10], F32, name="o_all")
            t2_all = tail.tile([128, 8, 10], F32, name="t2_all")
            # o = yn*inv + w1*qn + b2   (stride-0 broadcasts)
            nc.vector.tensor_tensor(o_all, yn_all,
                                    bc(inv[:, :, 0], 10, 2), op=ALU.mult)
            nc.vector.tensor_tensor(t2_all, bc(w1_bc, 8, 1),
                                    bc(qn[:, :, 0], 10, 2), op=ALU.mult)
            nc.vector.tensor_add(o_all, o_all, t2_all)
            nc.vector.tensor_add(o_all, o_all, bc(b2_bc, 8, 1))
            nc.sync.dma_start(out=out.rearrange("(t p) o -> p t o", p=128),
                              in_=o_all)
            tp2ctx.close()

    nc.compile()
    return nc


_NC_CACHE = None


def _get_program():
    global _NC_CACHE
    if _NC_CACHE is None:
        _NC_CACHE = _build_program()
    return _NC_CACHE


def _prepare_in_maps(inputs):
    x = np.asarray(inputs["x"], dtype=np.float32)
    w_enc = np.asarray(inputs["W_enc"], dtype=np.float32)
    w_res = np.asarray(inputs["W_res"], dtype=np.float32)
    w_in = np.asarray(inputs["W_in"], dtype=np.float32)
    bias = np.asarray(inputs["bias"], dtype=np.float32)
    ln_g = np.asarray(inputs["ln_g"], dtype=np.float32)
    ln_b = np.asarray(inputs["ln_b"], dtype=np.float32)
    w_out = np.asarray(inputs["W_out"], dtype=np.float32)
    b_out = np.asarray(inputs["b_out"], dtype=np.float32)

    w_c = (w_enc.T.astype(np.float64) @ w_in.astype(np.float64))
    w2 = w_out * ln_g[None, :]                       # [10, D]

    # encoder weights: 16*W_c in bf16, padded to 896 k-rows, layout [p, k, m]
    wcp = np.zeros((KE * 128, D), np.float64)
    wcp[:KX] = SW * w_c
    wc16 = np.ascontiguousarray(
        wcp.astype(ml_dtypes.bfloat16).reshape(KE, 128, D).transpose(1, 0, 2))

    bias16 = np.ascontiguousarray((SW * bias).reshape(KD, 128).T.astype(np.float32))

    # fp8 recurrent weights, upscaled by SW, layout [p, ksub, m]
    w8 = (SW * w_res).astype(ml_dtypes.float8_e4m3)
    w8 = np.ascontiguousarray(w8.reshape(KD, 128, D).transpose(1, 0, 2))

    # readout: [0.2*W2.T | ones] in bf16 hi + bf16 residual, layout [p, k, o]
    a = np.empty((D, 11), np.float64)
    a[:, :10] = DT_STEP * w2.T.astype(np.float64)
    a[:, 10] = 1.0
    a16 = a.astype(ml_dtypes.bfloat16)
    ar16 = (a - a16.astype(np.float64)).astype(ml_dtypes.bfloat16)
    a16 = np.ascontiguousarray(a16.reshape(KD, 128, 11).transpose(1, 0, 2))
    ar16 = np.ascontiguousarray(ar16.reshape(KD, 128, 11).transpose(1, 0, 2))

    w1v = w2.sum(axis=1).astype(np.float32)
    b2v = (w_out.astype(np.float64) @ ln_b.astype(np.float64)
           + b_out.astype(np.float64)).astype(np.float32)

    shared = {
        "wc": wc16,
        "w8": w8,
        "bias": bias16,
        "w2a": a16,
        "w2r": ar16,
        "w1": np.ascontiguousarray(w1v),
        "b2": np.ascontiguousarray(b2v),
    }
    # x pretransposed + bf16 on host (input marshalling), layout [p, k, b]
    xp = np.zeros((KE * 128, B), ml_dtypes.bfloat16)
    xp[:KX] = x.T.astype(ml_dtypes.bfloat16)
    xp = xp.reshape(KE, 128, B)
    in_maps = []
    for c in range(N_CORES):
        m = dict(shared)
        m["xt"] = np.ascontiguousarray(
            xp[:, :, c * R:(c + 1) * R].transpose(1, 0, 2))
        in_maps.append(m)
    return in_maps


def run(inputs, trace=False, tmpdir=None):
    """Run on 8 NeuronCores; returns (out [8192,10], BassKernelResults)."""
    nc = _get_program()
    in_maps = _prepare_in_maps(inputs)
    res = bass_utils.run_bass_kernel_spmd(
        nc, in_maps, core_ids=list(range(N_CORES)), trace=trace, tmpdir=tmpdir)
    outs = [np.asarray(r["out"]) for r in res.results]
    return np.concatenate(outs, axis=0), res


def kernel(**inputs):
    out, _ = run(inputs, trace=False)
    return out


# revision 47
# speedup vs baseline: 1.0102x; 1.0102x over previous
"""Trainium2 Bass kernel for nn_ChimeraNet (encoder -> 10-step Euler RNN -> LN -> readout).

Data-parallel over 8 NeuronCores: each core gets 1024 rows of the batch and a
replicated set of (host-prefolded) weights.

Math (per core, R=1024 rows, D=1024), in "drive space" z = h @ W_res + c:
    c   = x @ W_c + bias               with W_c = W_enc.T @ W_in (host-folded)
    z_0 = c;  T_k = tanh(z_k)
    z_{k+1} = 0.8 z_k + 0.2 c + 0.2 (T_k @ W_res)      k = 0..8
    u_{k+1} = 0.8 u_k + T_k                            k = 0..9,  u_0 = T_0
    h = 0.2 u_10;  out = LayerNorm(h) @ W_out.T + b_out (folded)

The z state is kept in the exponentially rescaled+upscaled frame
G_k = 16 z_k / 0.8^k (bf16) so each step's state update is a single
one-scalar DVE op reading the matmul PSUM directly:
    G_{k+1} = G_k + 1.25^{k+1} * psum
    psum    = 16 c (bf16 identity matmul) + T8 @ fp8(16 W_res)  (DoubleRow fp8)
    T_k     = tanh((0.8^k/16) * G_k)   (ACT with scale, fp8 out - no cast op)
The drive tiles store 16c in bf16 (the host folds the 16 into W_c), so the
identity matmuls run at bf16 rate and G_0 IS the drive tile.

Work distribution per Euler step (per-core, per [128,1024] tile x8):
    PE   : bf16 identity (re-add 16c) + 4x fp8-DR matmuls       ~17.4 us
    DVE  : 8 G-updates (STT from PSUM) + 5 u-updates (STT)      ~16 us
    ACT  : 8 fp8 tanhs + 5 exact bf16 tanhs for the DVE u's     ~14.7 us
    Pool : 3 u pre-decays + 3 u adds (m5-7, reading fp8 tanh)   ~14 us
Each step runs as 5 phases (m0 | m1 | m2,m3 | m4,m5 | m6,m7) over a 4-deep
[128,1024] psum rotation; splitting the first pair makes G0 (and with it the
13-op ACT chain the next step's matmuls wait on) start ~4us earlier.  The T
tiles are 4 per-k-pair fp8 tiles (own semaphores: a matmul waits only on the
2 tanhs it reads), double-buffered by step parity, and each phase's j-group
order is rotated to match when the previous step's tanhs land.  The last
step is Pool-free, processes tiles in reverse, and the readout matmuls sweep
k in that completion order; LN + readout run as batched [128,8]-wide ops on
a stacked transpose target with stride-0 broadcasts.

fp8 e4m3 is used for the recurrent matmul operands (T8 = direct fp8 tanh;
W8 = fp8(16 W_res)) and for the Pool-side u accumulation; the final tanh
T_9 (u-weight 1) and the 5 DVE-side u tiles use exact bf16 tanh, keeping
the final relative error ~1.1e-2.
"""

import os
import sys

import numpy as np

try:
    import concourse.bass as bass  # noqa: F401
except ImportError:  # pragma: no cover - fresh grading env without PYTHONPATH
    for p in ("/root/.axon_site", "/root/.axon_site/_ro/trn_rl_repo",
              "/root/.axon_site/_ro/pypackages", "/opt/trn_rl_repo"):
        if os.path.isdir(p) and p not in sys.path:
            sys.path.append(p)
    import concourse.bass as bass

from contextlib import ExitStack

import ml_dtypes
import concourse.tile as tile
from concourse import bacc, bass_utils, mybir
from concourse.masks import make_identity

N_CORES = 8
B = 8192
R = B // N_CORES        # rows per core
D = 1024                # latent dim
KX = 784                # encoder input dim
KE = 7                  # padded encoder k tiles (896 = 7*128)
DT_STEP = 0.2
STEPS = 10
EPS = 1e-5
SW = 16.0               # fp8 weight upscale (exact in bf16/f32)

F32 = mybir.dt.float32
BF16 = mybir.dt.bfloat16
F8 = mybir.dt.float8e4
AF = mybir.ActivationFunctionType
ALU = mybir.AluOpType
DR = mybir.MatmulPerfMode.DoubleRow

KD = D // 128           # 8 k/m tiles over D
NS = R // 512           # 2 moving-dim slices of 512 (psum bank width)
NWARM = 16              # PE warmup matmuls (hold clock while DMAs land)


def _build_program():
    nc = bacc.Bacc("TRN2", target_bir_lowering=False, debug=False)

    xt = nc.dram_tensor("xt", [128, KE, R], BF16, kind="ExternalInput").ap()
    wc = nc.dram_tensor("wc", [128, KE, D], BF16, kind="ExternalInput").ap()
    w8 = nc.dram_tensor("w8", [128, KD, D], F8, kind="ExternalInput").ap()
    bias = nc.dram_tensor("bias", [128, KD], F32, kind="ExternalInput").ap()
    w2a = nc.dram_tensor("w2a", [128, KD, 11], BF16, kind="ExternalInput").ap()
    w2r = nc.dram_tensor("w2r", [128, KD, 11], BF16, kind="ExternalInput").ap()
    w1 = nc.dram_tensor("w1", [10], F32, kind="ExternalInput").ap()
    b2 = nc.dram_tensor("b2", [10], F32, kind="ExternalInput").ap()
    out = nc.dram_tensor("out", [R, 10], F32, kind="ExternalOutput").ap()

    with tile.TileContext(nc) as tc, ExitStack() as ctx:
        state = ctx.enter_context(tc.tile_pool(name="state", bufs=1))
        consts = ctx.enter_context(tc.tile_pool(name="consts", bufs=1))
        wres_pool = ctx.enter_context(tc.tile_pool(name="wres", bufs=1))

        # persistent SBUF state (G in fp32 updated in place, u in bf16,
        # drive holds 16c in bf16)
        g = [state.tile([128, R], BF16, name=f"g{k}", tag=f"g{k}") for k in range(KD)]
        u = [state.tile([128, R], BF16, name=f"u{k}", tag=f"u{k}") for k in range(KD)]
        drive = [state.tile([128, R], BF16, name=f"dr{k}", tag=f"dr{k}")
                 for k in range(KD)]
        # T in fp8, split into per-k-pair tiles (own semaphores -> matmuls
        # wait only on the 2 casts they read) and double-buffered by step
        # parity (no WAR between step s's casts and step s's matmuls).
        t8p = [[state.tile([128, 2, R], F8, name=f"t8_{b}_{jj}", tag=f"t8_{b}_{jj}")
                for jj in range(4)] for b in range(2)]
        w8_sb = wres_pool.tile([128, KD, D], F8, name="w8", tag="w8")

        with ExitStack() as mmctx:
            # one psum pool: 4 x [128,1024] f32 = all 8 banks
            psum = mmctx.enter_context(
                tc.tile_pool(name="mm", bufs=4, space="PSUM"))

            # input DMAs first (queues fill while PE warms up)
            xt_pool = ctx.enter_context(tc.tile_pool(name="xt", bufs=1))
            wc_pool = ctx.enter_context(tc.tile_pool(name="wc", bufs=1))
            xt_sb = xt_pool.tile([128, KE, R], BF16, name="xt")
            wc_sb = wc_pool.tile([128, KE, D], BF16, name="wc")
            # encoder-critical 3.5MB (xt + wc) balanced ~1.17MB across all
            # three DGE trigger queues; w8 follows on gpsimd (needed only
            # when the Euler loop starts).
            nc.sync.dma_start(out=xt_sb[:, :4, :], in_=xt[:, :4, :])
            nc.gpsimd.dma_start(out=xt_sb[:, 4:, :], in_=xt[:, 4:, :])
            nc.gpsimd.dma_start(out=wc_sb[:, 5:, :], in_=wc[:, 5:, :])
            nc.scalar.dma_start(out=wc_sb[:, :5, :], in_=wc[:, :5, :])
            bias_sb = consts.tile([128, KD], F32)
            nc.gpsimd.dma_start(out=bias_sb, in_=bias)
            nc.gpsimd.dma_start(out=w8_sb, in_=w8)

            # tail weights (tiny, same cheap gpsimd queue)
            tail = ctx.enter_context(tc.tile_pool(name="tail", bufs=1))
            w2a_sb = tail.tile([128, KD, 11], BF16)
            nc.gpsimd.dma_start(out=w2a_sb, in_=w2a)
            w2r_sb = tail.tile([128, KD, 11], BF16)
            nc.gpsimd.dma_start(out=w2r_sb, in_=w2r)
            w1_bc = tail.tile([128, 10], F32)
            nc.gpsimd.dma_start(out=w1_bc, in_=bass.AP(tensor=w1.tensor, offset=w1.offset,
                                                       ap=[[0, 128]] + list(w1.ap)))
            b2_bc = tail.tile([128, 10], F32)
            nc.gpsimd.dma_start(out=b2_bc, in_=bass.AP(tensor=b2.tensor, offset=b2.offset,
                                                       ap=[[0, 128]] + list(b2.ap)))

            # PE warmup: dependency-free f32 matmuls pull the clock gate to
            # full speed while the input DMAs are in flight.
            warm_src = consts.tile([128, 512], F32)
            nc.vector.memset(warm_src, 0.01)
            warm_sb = consts.tile([128, 1], F32)
            for w in range(NWARM):
                wp = psum.tile([128, 512], F32, name=f"warm{w}", tag="mm")
                nc.tensor.matmul(wp, lhsT=warm_src[:, :128], rhs=warm_src,
                                 start=True, stop=True)
                if w == NWARM - 1:
                    nc.vector.tensor_copy(warm_sb, wp[:, :1])  # keep-alive

            ident = consts.tile([128, 128], F32)
            make_identity(nc, ident)
            ident16 = consts.tile([128, 128], BF16)
            nc.vector.tensor_copy(ident16, ident)
            # broadcast 0.8 tile: lets the Pool engine do u *= 0.8 as a plain
            # tensor_tensor (Pool supports neither STT nor tensor_scalar)
            decay_sb = consts.tile([128, R], BF16)
            nc.vector.memset(decay_sb, 1.0 - DT_STEP)

            tau_pool = ctx.enter_context(tc.tile_pool(name="tau", bufs=6))

            # ------------ encoder: 16c = xT.T @ (16 W_c) + 16 bias ----------
            # m-outer so each m's eviction+prologue hides under the next m's
            # matmul sweep.
            for m in range(KD):
                ps = psum.tile([128, R], F32, name=f"eps{m}", tag="mm")
                for n in range(NS):
                    sl = slice(n * 512, (n + 1) * 512)
                    for k in range(KE):
                        nc.tensor.matmul(
                            ps[:, sl],
                            lhsT=wc_sb[:, k, m * 128:(m + 1) * 128],
                            rhs=xt_sb[:, k, sl],
                            start=(k == 0), stop=(k == KE - 1))
                # evict 16c + 16bias -> bf16 drive (G_0 frame)
                nc.scalar.activation(drive[m], ps, AF.Identity,
                                     bias=bias_sb[:, m:m + 1], scale=1.0)
                # prologue: T_0 = tanh(z_0) straight to fp8 (G_0 = 16c);
                # u_0 = T_0 via fp8->bf16 copy
                nc.scalar.activation(t8p[0][m // 2][:, m % 2, :], drive[m],
                                     AF.Tanh, scale=float(1.0 / SW))
                nc.vector.tensor_copy(u[m], t8p[0][m // 2][:, m % 2, :])

            sqp = ctx.enter_context(tc.tile_pool(name="sq", bufs=1))
            sq_tiles = [sqp.tile([128, R], BF16, name=f"sq{k}", tag=f"sq{k}")
                        for k in range(KD)]

            # ------------ Euler integration loop (16z/0.8^k frame) ----------
            def mm_id(ps, m):
                for n in range(NS):
                    sl = slice(n * 512, (n + 1) * 512)
                    nc.tensor.matmul(ps[:, sl], lhsT=ident16,
                                     rhs=drive[m][:, sl],
                                     start=True, stop=False)

            def mm_f8(ps, m, j, stop, rbuf):
                lhsT = w8_sb[:, 2 * j:2 * j + 2, m * 128:(m + 1) * 128]
                for n in range(NS):
                    sl = slice(n * 512, (n + 1) * 512)
                    nc.tensor.matmul(ps[:, sl], lhsT=lhsT,
                                     rhs=t8p[rbuf][j][:, :, sl],
                                     perf_mode=DR,
                                     start=False, stop=stop)

            # Per step: 4 pair-phases (2 m-tiles each, psum 4-buf rotation).
            # Pair p runs its fp8 j-groups in rotated order ending with
            # k-pair p, so no matmul ever waits on the previous step's late
            # tanh/casts (k6/k7 feed pair2's FIRST group, issued ~9us in).
            # DVE interleaves u-updates into its psum-wait bubbles.
            def u_upd(s, m, tau, last):
                # u_{s+2} = 0.8 u_{s+1} + T_{s+1}; m0-4 on DVE (STT), m5-7 on
                # Pool (pre-decayed tensor add). tau is the fp8 tanh slice
                # except on the last step (exact bf16 -- T_9 has u-weight 1).
                # Last step avoids Pool entirely: its slow, late-draining
                # queue would otherwise pace the readout.
                if m < 5 or last:
                    nc.vector.scalar_tensor_tensor(
                        u[m], in0=u[m], scalar=1.0 - DT_STEP,
                        in1=tau, op0=ALU.mult, op1=ALU.add)
                else:
                    nc.gpsimd.tensor_add(u[m], u[m], tau)
                if last:
                    nc.vector.tensor_mul(sq_tiles[m], u[m], u[m])

            for s in range(STEPS - 1):
                ak1 = float(0.8 ** (s + 1) / SW)       # tanh scale, step s+1
                qk = float(DT_STEP * 1.25 ** (s + 1))  # G-update scalar
                cur = drive if s == 0 else g
                last = (s + 1 == STEPS - 1)
                rbuf, wbuf = s % 2, (s + 1) % 2
                taus = [None] * KD
                # Normal steps split the first pair into single-m phases so
                # G0 (and with it the whole ACT tanh chain) starts ~4us
                # earlier -- the next step's matmuls wait on that chain.
                # The last step instead processes Pool's tiles (m5-7) first
                # so the tail's readout pacing starts on finished tiles.
                # per-phase fp8 j-group order, matched to when the
                # previous step's tanhs land (t8p[3] is the latest)
                jorders = [[1, 2, 0, 3], [2, 0, 1, 3],
                           [0, 1, 2, 3], [1, 2, 3, 0], [2, 3, 0, 1],
                           [0, 1, 2, 3]]
                if last:
                    # singles at both ends: m6 early starts the readout
                    # pacing; m0/m1 singles land the final u/sq sooner
                    phases = [(6,), (7,), (0,), (1,), (4, 5), (2, 3)]
                else:
                    phases = [(0,), (1,), (2, 3), (4, 5), (6, 7)]
                prev_ms = None
                # Pool pre-decay for its u tiles (no deps on this step's taus)
                if not last:
                    for m in range(5, KD):
                        nc.gpsimd.tensor_mul(u[m], u[m], decay_sb)
                for q, ms in enumerate(phases):
                    pss = {}
                    for m in ms:
                        pss[m] = psum.tile([128, R], F32, name=f"ps{s}_{m}",
                                           tag="mm")
                        mm_id(pss[m], m)
                    jorder = jorders[q]
                    for jpos, j in enumerate(jorder):
                        for m in ms:
                            mm_f8(pss[m], m, j, stop=(jpos == 3), rbuf=rbuf)
                    for m in ms:
                        nc.vector.scalar_tensor_tensor(
                            g[m], in0=pss[m], scalar=qk,
                            in1=cur[m], op0=ALU.mult, op1=ALU.add)
                    # fp8 tanhs first (they feed the next step's matmuls);
                    # the bf16 double-tanhs for the DVE u-STTs have slack
                    for m in ms:
                        if last:
                            # final tanh in exact bf16 for the u accumulator
                            tau = tau_pool.tile([128, R], BF16,
                                                name=f"tau9_{m}", tag="tau")
                            nc.scalar.activation(tau, g[m], AF.Tanh, scale=ak1)
                            taus[m] = tau
                        else:
                            # tanh straight to fp8 (no separate cast op)
                            dst = t8p[wbuf][m // 2][:, m % 2, :]
                            nc.scalar.activation(dst, g[m], AF.Tanh, scale=ak1)
                            taus[m] = dst
                    if not last:
                        # second, exact bf16 tanhs for the DVE u-STTs (fp8
                        # in1 runs ~2.5x slower on DVE; ACT has slack and u
                        # gets full tanh precision). b3/b4 are deferred
                        # behind f6/f7 so the last matmul-gating fp8 tanhs
                        # land ~2us earlier in ACT's queue; their consumers
                        # (u3/u4) run at the end of the DVE queue.
                        bms = [m for m in ms if m < 3]
                        if q == len(phases) - 1:
                            bms.extend([3, 4])
                        for m in bms:
                            tau = tau_pool.tile([128, R], BF16,
                                                name=f"tau{s + 1}_{m}",
                                                tag="tau")
                            nc.scalar.activation(tau, g[m], AF.Tanh,
                                                 scale=ak1)
                            taus[m] = tau
                    # u-updates for the PREVIOUS phase slot into DVE bubbles
                    # (u3 deferred to the end alongside its late tanh)
                    if prev_ms is not None:
                        for m in prev_ms:
                            if last or m != 3:
                                u_upd(s, m, taus[m], last)
                    prev_ms = ms
                for m in prev_ms:
                    u_upd(s, m, taus[m], last)
                if not last:
                    u_upd(s, 3, taus[3], last)

            # ------------ tail: LN stats + readout (matmul part) ------------
            ones_sb = tail.tile([128, 1], BF16)
            nc.vector.memset(ones_sb, 1.0)
            eps_sb = tail.tile([128, 1], F32)
            nc.vector.memset(eps_sb, EPS)

            s2_sb = tail.tile([1, R], F32)
            y_sb = tail.tile([11, R], F32)

            # y matmuls first (paced by u finalization: the last Euler step
            # processes pairs in reverse, so sweep k in that completion
            # order), s2 after (paced by the sq tiles).
            KORD = [6, 7, 0, 1, 4, 5, 2, 3]
            yps = [psum.tile([11, 512], F32, name=f"yp{n}", tag="mm")
                   for n in range(NS)]
            for ki, k in enumerate(KORD):
                for n in range(NS):
                    sl = slice(n * 512, (n + 1) * 512)
                    nc.tensor.matmul(yps[n], lhsT=w2a_sb[:, k, :],
                                     rhs=u[k][:, sl],
                                     start=(ki == 0), stop=False)
            s2s = [psum.tile([1, 512], F32, name=f"s2p{n}", tag="mm")
                   for n in range(NS)]
            for ki, k in enumerate(KORD):
                for n in range(NS):
                    sl = slice(n * 512, (n + 1) * 512)
                    nc.tensor.matmul(s2s[n], lhsT=ones_sb,
                                     rhs=sq_tiles[k][:, sl],
                                     start=(ki == 0), stop=(ki == KD - 1))
            for n in range(NS):
                nc.scalar.copy(s2_sb[:, n * 512:(n + 1) * 512], s2s[n])
            for ki, k in enumerate(KORD):
                for n in range(NS):
                    sl = slice(n * 512, (n + 1) * 512)
                    nc.tensor.matmul(yps[n], lhsT=w2r_sb[:, k, :],
                                     rhs=u[k][:, sl],
                                     start=False, stop=(ki == KD - 1))
            for n in range(NS):
                nc.scalar.copy(y_sb[:, n * 512:(n + 1) * 512], yps[n])

            mmctx.close()

            def bc(ap, n, axis):
                # broadcast an AP along a new stride-0 dim inserted at `axis`
                newap = list(ap.ap)
                newap.insert(axis, [0, n])
                return bass.AP(tensor=ap.tensor, offset=ap.offset, ap=newap)

            # batched LN + readout: transpose all 8 row-tiles into one stacked
            # [128, rt, 12] psum tile, then do the whole LN/readout chain as
            # [128,8]-wide ops instead of 8 serial per-rt chains.
            tp2ctx = ExitStack()
            tp2 = tp2ctx.enter_context(
                tc.tile_pool(name="tp2", bufs=1, space="PSUM"))
            tp_all = tp2.tile([128, 8, 12], F32, name="tp_all")
            for rt in range(8):
                sl = slice(rt * 128, (rt + 1) * 128)
                nc.tensor.transpose(tp_all[:, rt, 0:11], y_sb[:, sl],
                                    ident[:11, :11])
                nc.tensor.transpose(tp_all[:, rt, 11:12], s2_sb[:, sl],
                                    ident[:1, :1])
            st_all = tail.tile([128, 8, 12], F32, name="st_all")
            nc.vector.tensor_copy(st_all, tp_all)
            yn_all = st_all[:, :, 0:10]
            mu_n = tail.tile([128, 8, 1], F32, name="mu_all")
            nc.scalar.mul(mu_n, st_all[:, :, 10:11], -DT_STEP / D)  # -mean(h)
            ex2 = tail.tile([128, 8, 1], F32, name="ex2_all")
            nc.scalar.mul(ex2, st_all[:, :, 11:12], DT_STEP * DT_STEP / D)
            var = tail.tile([128, 8, 1], F32, name="var_all")
            nc.vector.scalar_tensor_tensor(var, in0=mu_n, scalar=-1.0,
                                           op0=ALU.mult, in1=mu_n,
                                           op1=ALU.mult)      # -mean^2
            nc.vector.tensor_add(var, var, ex2)
            sd = tail.tile([128, 8, 1], F32, name="sd_all")
            nc.scalar.activation(sd, var, AF.Sqrt, bias=eps_sb, scale=1.0)
            inv = tail.tile([128, 8, 1], F32, name="inv_all")
            nc.vector.reciprocal(inv, sd)
            qn = tail.tile([128, 8, 1], F32, name="qn_all")
            nc.vector.tensor_mul(qn, mu_n, inv)                     # -mu*inv

            o_all = tail.tile([128, 8, 10], F32, name="o_all")
            t2_all = tail.tile([128, 8, 10], F32, name="t2_all")
            # o = yn*inv + w1*qn + b2   (stride-0 broadcasts)
            nc.vector.tensor_tensor(o_all, yn_all,
                                    bc(inv[:, :, 0], 10, 2), op=ALU.mult)
            nc.vector.tensor_tensor(t2_all, bc(w1_bc, 8, 1),
                                    bc(qn[:, :, 0], 10, 2), op=ALU.mult)
            nc.vector.tensor_add(o_all, o_all, t2_all)
            nc.vector.tensor_add(o_all, o_all, bc(b2_bc, 8, 1))
            nc.sync.dma_start(out=out.rearrange("(t p) o -> p t o", p=128),
                              in_=o_all)
            tp2ctx.close()

    nc.compile()
    return nc


_NC_CACHE = None


def _get_program():
    global _NC_CACHE
    if _NC_CACHE is None:
        _NC_CACHE = _build_program()
    return _NC_CACHE


def _prepare_in_maps(inputs):
    x = np.asarray(inputs["x"], dtype=np.float32)
    w_enc = np.asarray(inputs["W_enc"], dtype=np.float32)
    w_res = np.asarray(inputs["W_res"], dtype=np.float32)
    w_in = np.asarray(inputs["W_in"], dtype=np.float32)
    bias = np.asarray(inputs["bias"], dtype=np.float32)
    ln_g = np.asarray(inputs["ln_g"], dtype=np.float32)
    ln_b = np.asarray(inputs["ln_b"], dtype=np.float32)
    w_out = np.asarray(inputs["W_out"], dtype=np.float32)
    b_out = np.asarray(inputs["b_out"], dtype=np.float32)

    w_c = (w_enc.T.astype(np.float64) @ w_in.astype(np.float64))
    w2 = w_out * ln_g[None, :]                       # [10, D]

    # encoder weights: 16*W_c in bf16, padded to 896 k-rows, layout [p, k, m]
    wcp = np.zeros((KE * 128, D), np.float64)
    wcp[:KX] = SW * w_c
    wc16 = np.ascontiguousarray(
        wcp.astype(ml_dtypes.bfloat16).reshape(KE, 128, D).transpose(1, 0, 2))

    bias16 = np.ascontiguousarray((SW * bias).reshape(KD, 128).T.astype(np.float32))

    # fp8 recurrent weights, upscaled by SW, layout [p, ksub, m]
    w8 = (SW * w_res).astype(ml_dtypes.float8_e4m3)
    w8 = np.ascontiguousarray(w8.reshape(KD, 128, D).transpose(1, 0, 2))

    # readout: [0.2*W2.T | ones] in bf16 hi + bf16 residual, layout [p, k, o]
    a = np.empty((D, 11), np.float64)
    a[:, :10] = DT_STEP * w2.T.astype(np.float64)
    a[:, 10] = 1.0
    a16 = a.astype(ml_dtypes.bfloat16)
    ar16 = (a - a16.astype(np.float64)).astype(ml_dtypes.bfloat16)
    a16 = np.ascontiguousarray(a16.reshape(KD, 128, 11).transpose(1, 0, 2))
    ar16 = np.ascontiguousarray(ar16.reshape(KD, 128, 11).transpose(1, 0, 2))

    w1v = w2.sum(axis=1).astype(np.float32)
    b2v = (w_out.astype(np.float64) @ ln_b.astype(np.float64)
           + b_out.astype(np.float64)).astype(np.float32)

    shared = {
        "wc": wc16,
        "w8": w8,
        "bias": bias16,
        "w2a": a16,
        "w2r": ar16,
        "w1": np.ascontiguousarray(w1v),
        "b2": np.ascontiguousarray(b2v),
    }
    # x pretransposed + bf16 on host (input marshalling), layout [p, k, b]
    xp = np.zeros((KE * 128, B), ml_dtypes.bfloat16)
    xp[:KX] = x.T.astype(ml_dtypes.bfloat16)
    xp = xp.reshape(KE, 128, B)
    in_maps = []
    for c in range(N_CORES):
        m = dict(shared)
        m["xt"] = np.ascontiguousarray(
            xp[:, :, c * R:(c + 1) * R].transpose(1, 0, 2))
        in_maps.append(m)
    return in_maps


def run(inputs, trace=False, tmpdir=None):
    """Run on 8 NeuronCores; returns (out [8192,10], BassKernelResults)."""
    nc = _get_program()
    in_maps = _prepare_in_maps(inputs)
    res = bass_utils.run_bass_kernel_spmd(
        nc, in_maps, core_ids=list(range(N_CORES)), trace=trace, tmpdir=tmpdir)
    outs = [np.asarray(r["out"]) for r in res.results]
    return np.concatenate(outs, axis=0), res


def kernel(**inputs):
    out, _ = run(inputs, trace=False)
    return out


# revision 48
# speedup vs baseline: 1.0128x; 1.0025x over previous
"""Trainium2 Bass kernel for nn_ChimeraNet (encoder -> 10-step Euler RNN -> LN -> readout).

Data-parallel over 8 NeuronCores: each core gets 1024 rows of the batch and a
replicated set of (host-prefolded) weights.

Math (per core, R=1024 rows, D=1024), in "drive space" z = h @ W_res + c:
    c   = x @ W_c + bias               with W_c = W_enc.T @ W_in (host-folded)
    z_0 = c;  T_k = tanh(z_k)
    z_{k+1} = 0.8 z_k + 0.2 c + 0.2 (T_k @ W_res)      k = 0..8
    u_{k+1} = 0.8 u_k + T_k                            k = 0..9,  u_0 = T_0
    h = 0.2 u_10;  out = LayerNorm(h) @ W_out.T + b_out (folded)

The z state is kept in the exponentially rescaled+upscaled frame
G_k = 16 z_k / 0.8^k (bf16) so each step's state update is a single
one-scalar DVE op reading the matmul PSUM directly:
    G_{k+1} = G_k + 1.25^{k+1} * psum
    psum    = 16 c (bf16 identity matmul) + T8 @ fp8(16 W_res)  (DoubleRow fp8)
    T_k     = tanh((0.8^k/16) * G_k)   (ACT with scale, fp8 out - no cast op)
The drive tiles store 16c in bf16 (the host folds the 16 into W_c), so the
identity matmuls run at bf16 rate and G_0 IS the drive tile.

Work distribution per Euler step (per-core, per [128,1024] tile x8):
    PE   : bf16 identity (re-add 16c) + 4x fp8-DR matmuls       ~17.4 us
    DVE  : 8 G-updates (STT from PSUM) + 5 u-updates (STT)      ~16 us
    ACT  : 8 fp8 tanhs + 5 exact bf16 tanhs for the DVE u's     ~14.7 us
    Pool : 3 u pre-decays + 3 u adds (m5-7, reading fp8 tanh)   ~14 us
Each step runs as 5 phases (m0 | m1 | m2,m3 | m4,m5 | m6,m7) over a 4-deep
[128,1024] psum rotation; splitting the first pair makes G0 (and with it the
13-op ACT chain the next step's matmuls wait on) start ~4us earlier.  The T
tiles are 4 per-k-pair fp8 tiles (own semaphores: a matmul waits only on the
2 tanhs it reads), double-buffered by step parity, and each phase's j-group
order is rotated to match when the previous step's tanhs land.  The last
step is Pool-free, processes tiles in reverse, and the readout matmuls sweep
k in that completion order; LN + readout run as batched [128,8]-wide ops on
a stacked transpose target with stride-0 broadcasts.

fp8 e4m3 is used for the recurrent matmul operands (T8 = direct fp8 tanh;
W8 = fp8(16 W_res)) and for the Pool-side u accumulation; the final tanh
T_9 (u-weight 1) and the 5 DVE-side u tiles use exact bf16 tanh, keeping
the final relative error ~1.1e-2.
"""

import os
import sys

import numpy as np

try:
    import concourse.bass as bass  # noqa: F401
except ImportError:  # pragma: no cover - fresh grading env without PYTHONPATH
    for p in ("/root/.axon_site", "/root/.axon_site/_ro/trn_rl_repo",
              "/root/.axon_site/_ro/pypackages", "/opt/trn_rl_repo"):
        if os.path.isdir(p) and p not in sys.path:
            sys.path.append(p)
    import concourse.bass as bass

from contextlib import ExitStack

import ml_dtypes
import concourse.tile as tile
from concourse import bacc, bass_utils, mybir
from concourse.masks import make_identity

N_CORES = 8
B = 8192
R = B // N_CORES        # rows per core
D = 1024                # latent dim
KX = 784                # encoder input dim
KE = 7                  # padded encoder k tiles (896 = 7*128)
DT_STEP = 0.2
STEPS = 10
EPS = 1e-5
SW = 16.0               # fp8 weight upscale (exact in bf16/f32)

F32 = mybir.dt.float32
BF16 = mybir.dt.bfloat16
F8 = mybir.dt.float8e4
AF = mybir.ActivationFunctionType
ALU = mybir.AluOpType
DR = mybir.MatmulPerfMode.DoubleRow

KD = D // 128           # 8 k/m tiles over D
NS = R // 512           # 2 moving-dim slices of 512 (psum bank width)
NWARM = 16              # PE warmup matmuls (hold clock while DMAs land)


def _build_program():
    nc = bacc.Bacc("TRN2", target_bir_lowering=False, debug=False)

    xt = nc.dram_tensor("xt", [128, KE, R], BF16, kind="ExternalInput").ap()
    wc = nc.dram_tensor("wc", [128, KE, D], BF16, kind="ExternalInput").ap()
    w8 = nc.dram_tensor("w8", [128, KD, D], F8, kind="ExternalInput").ap()
    bias = nc.dram_tensor("bias", [128, KD], F32, kind="ExternalInput").ap()
    w2a = nc.dram_tensor("w2a", [128, KD, 11], BF16, kind="ExternalInput").ap()
    w2r = nc.dram_tensor("w2r", [128, KD, 11], BF16, kind="ExternalInput").ap()
    w1 = nc.dram_tensor("w1", [10], F32, kind="ExternalInput").ap()
    b2 = nc.dram_tensor("b2", [10], F32, kind="ExternalInput").ap()
    out = nc.dram_tensor("out", [R, 10], F32, kind="ExternalOutput").ap()

    with tile.TileContext(nc) as tc, ExitStack() as ctx:
        state = ctx.enter_context(tc.tile_pool(name="state", bufs=1))
        consts = ctx.enter_context(tc.tile_pool(name="consts", bufs=1))
        wres_pool = ctx.enter_context(tc.tile_pool(name="wres", bufs=1))

        # persistent SBUF state (G in fp32 updated in place, u in bf16,
        # drive holds 16c in bf16)
        g = [state.tile([128, R], BF16, name=f"g{k}", tag=f"g{k}") for k in range(KD)]
        u = [state.tile([128, R], BF16, name=f"u{k}", tag=f"u{k}") for k in range(KD)]
        drive = [state.tile([128, R], BF16, name=f"dr{k}", tag=f"dr{k}")
                 for k in range(KD)]
        # T in fp8, split into per-k-pair tiles (own semaphores -> matmuls
        # wait only on the 2 casts they read) and double-buffered by step
        # parity (no WAR between step s's casts and step s's matmuls).
        t8p = [[state.tile([128, 2, R], F8, name=f"t8_{b}_{jj}", tag=f"t8_{b}_{jj}")
                for jj in range(4)] for b in range(2)]
        w8_sb = wres_pool.tile([128, KD, D], F8, name="w8", tag="w8")

        with ExitStack() as mmctx:
            # one psum pool: 4 x [128,1024] f32 = all 8 banks
            psum = mmctx.enter_context(
                tc.tile_pool(name="mm", bufs=4, space="PSUM"))

            # input DMAs first (queues fill while PE warms up)
            xt_pool = ctx.enter_context(tc.tile_pool(name="xt", bufs=1))
            wc_pool = ctx.enter_context(tc.tile_pool(name="wc", bufs=1))
            xt_sb = xt_pool.tile([128, KE, R], BF16, name="xt")
            wc_sb = wc_pool.tile([128, KE, D], BF16, name="wc")
            # encoder-critical 3.5MB (xt + wc) balanced ~1.17MB across all
            # three DGE trigger queues; w8 follows on gpsimd (needed only
            # when the Euler loop starts).
            nc.sync.dma_start(out=xt_sb[:, :4, :], in_=xt[:, :4, :])
            nc.gpsimd.dma_start(out=xt_sb[:, 4:, :], in_=xt[:, 4:, :])
            nc.gpsimd.dma_start(out=wc_sb[:, 5:, :], in_=wc[:, 5:, :])
            nc.scalar.dma_start(out=wc_sb[:, :5, :], in_=wc[:, :5, :])
            bias_sb = consts.tile([128, KD], F32)
            nc.gpsimd.dma_start(out=bias_sb, in_=bias)
            nc.gpsimd.dma_start(out=w8_sb, in_=w8)

            # tail weights (tiny, same cheap gpsimd queue)
            tail = ctx.enter_context(tc.tile_pool(name="tail", bufs=1))
            w2a_sb = tail.tile([128, KD, 11], BF16)
            nc.gpsimd.dma_start(out=w2a_sb, in_=w2a)
            w2r_sb = tail.tile([128, KD, 11], BF16)
            nc.gpsimd.dma_start(out=w2r_sb, in_=w2r)
            w1_bc = tail.tile([128, 10], F32)
            nc.gpsimd.dma_start(out=w1_bc, in_=bass.AP(tensor=w1.tensor, offset=w1.offset,
                                                       ap=[[0, 128]] + list(w1.ap)))
            b2_bc = tail.tile([128, 10], F32)
            nc.gpsimd.dma_start(out=b2_bc, in_=bass.AP(tensor=b2.tensor, offset=b2.offset,
                                                       ap=[[0, 128]] + list(b2.ap)))

            # PE warmup: dependency-free f32 matmuls pull the clock gate to
            # full speed while the input DMAs are in flight.
            warm_src = consts.tile([128, 512], F32)
            nc.vector.memset(warm_src, 0.01)
            warm_sb = consts.tile([128, 1], F32)
            for w in range(NWARM):
                wp = psum.tile([128, 512], F32, name=f"warm{w}", tag="mm")
                nc.tensor.matmul(wp, lhsT=warm_src[:, :128], rhs=warm_src,
                                 start=True, stop=True)
                if w == NWARM - 1:
                    nc.vector.tensor_copy(warm_sb, wp[:, :1])  # keep-alive

            ident = consts.tile([128, 128], F32)
            make_identity(nc, ident)
            ident16 = consts.tile([128, 128], BF16)
            nc.vector.tensor_copy(ident16, ident)
            # broadcast 0.8 tile: lets the Pool engine do u *= 0.8 as a plain
            # tensor_tensor (Pool supports neither STT nor tensor_scalar)
            decay_sb = consts.tile([128, R], BF16)
            nc.vector.memset(decay_sb, 1.0 - DT_STEP)

            tau_pool = ctx.enter_context(tc.tile_pool(name="tau", bufs=6))

            # ------------ encoder: 16c = xT.T @ (16 W_c) + 16 bias ----------
            # m-outer so each m's eviction+prologue hides under the next m's
            # matmul sweep.
            for m in range(KD):
                ps = psum.tile([128, R], F32, name=f"eps{m}", tag="mm")
                for n in range(NS):
                    sl = slice(n * 512, (n + 1) * 512)
                    for k in range(KE):
                        nc.tensor.matmul(
                            ps[:, sl],
                            lhsT=wc_sb[:, k, m * 128:(m + 1) * 128],
                            rhs=xt_sb[:, k, sl],
                            start=(k == 0), stop=(k == KE - 1))
                # evict 16c + 16bias -> bf16 drive (G_0 frame)
                nc.scalar.activation(drive[m], ps, AF.Identity,
                                     bias=bias_sb[:, m:m + 1], scale=1.0)
                # prologue: T_0 = tanh(z_0) straight to fp8 (G_0 = 16c);
                # u_0 = T_0 via fp8->bf16 copy
                nc.scalar.activation(t8p[0][m // 2][:, m % 2, :], drive[m],
                                     AF.Tanh, scale=float(1.0 / SW))
                nc.vector.tensor_copy(u[m], t8p[0][m // 2][:, m % 2, :])

            sqp = ctx.enter_context(tc.tile_pool(name="sq", bufs=1))
            sq_tiles = [sqp.tile([128, R], BF16, name=f"sq{k}", tag=f"sq{k}")
                        for k in range(KD)]

            # ------------ Euler integration loop (16z/0.8^k frame) ----------
            def mm_id(ps, m):
                for n in range(NS):
                    sl = slice(n * 512, (n + 1) * 512)
                    nc.tensor.matmul(ps[:, sl], lhsT=ident16,
                                     rhs=drive[m][:, sl],
                                     start=True, stop=False)

            def mm_f8(ps, m, j, stop, rbuf):
                lhsT = w8_sb[:, 2 * j:2 * j + 2, m * 128:(m + 1) * 128]
                for n in range(NS):
                    sl = slice(n * 512, (n + 1) * 512)
                    nc.tensor.matmul(ps[:, sl], lhsT=lhsT,
                                     rhs=t8p[rbuf][j][:, :, sl],
                                     perf_mode=DR,
                                     start=False, stop=stop)

            # Per step: 4 pair-phases (2 m-tiles each, psum 4-buf rotation).
            # Pair p runs its fp8 j-groups in rotated order ending with
            # k-pair p, so no matmul ever waits on the previous step's late
            # tanh/casts (k6/k7 feed pair2's FIRST group, issued ~9us in).
            # DVE interleaves u-updates into its psum-wait bubbles.
            def u_upd(s, m, tau, last):
                # u_{s+2} = 0.8 u_{s+1} + T_{s+1}; m0-4 on DVE (STT), m5-7 on
                # Pool (pre-decayed tensor add). tau is the fp8 tanh slice
                # except on the last step (exact bf16 -- T_9 has u-weight 1).
                # Last step avoids Pool entirely: its slow, late-draining
                # queue would otherwise pace the readout.
                if m < 5 or last:
                    nc.vector.scalar_tensor_tensor(
                        u[m], in0=u[m], scalar=1.0 - DT_STEP,
                        in1=tau, op0=ALU.mult, op1=ALU.add)
                else:
                    nc.gpsimd.tensor_add(u[m], u[m], tau)
                if last:
                    nc.vector.tensor_mul(sq_tiles[m], u[m], u[m])

            for s in range(STEPS - 1):
                ak1 = float(0.8 ** (s + 1) / SW)       # tanh scale, step s+1
                qk = float(DT_STEP * 1.25 ** (s + 1))  # G-update scalar
                cur = drive if s == 0 else g
                last = (s + 1 == STEPS - 1)
                rbuf, wbuf = s % 2, (s + 1) % 2
                taus = [None] * KD
                # Normal steps split the first pair into single-m phases so
                # G0 (and with it the whole ACT tanh chain) starts ~4us
                # earlier -- the next step's matmuls wait on that chain.
                # The last step instead processes Pool's tiles (m5-7) first
                # so the tail's readout pacing starts on finished tiles.
                # per-phase fp8 j-group order, matched to when the
                # previous step's tanhs land (t8p[3] is the latest)
                jorders = [[1, 0, 2, 3], [2, 0, 1, 3],
                           [0, 1, 2, 3], [1, 2, 3, 0], [2, 3, 0, 1],
                           [0, 1, 2, 3]]
                if last:
                    # singles at both ends: m6 early starts the readout
                    # pacing; m0/m1 singles land the final u/sq sooner
                    phases = [(6,), (7,), (0,), (1,), (4, 5), (2, 3)]
                else:
                    phases = [(0,), (1,), (2, 3), (4, 5), (6, 7)]
                prev_ms = None
                # Pool pre-decay for its u tiles (no deps on this step's taus)
                if not last:
                    for m in range(5, KD):
                        nc.gpsimd.tensor_mul(u[m], u[m], decay_sb)
                for q, ms in enumerate(phases):
                    pss = {}
                    for m in ms:
                        pss[m] = psum.tile([128, R], F32, name=f"ps{s}_{m}",
                                           tag="mm")
                        mm_id(pss[m], m)
                    jorder = jorders[q]
                    for jpos, j in enumerate(jorder):
                        for m in ms:
                            mm_f8(pss[m], m, j, stop=(jpos == 3), rbuf=rbuf)
                    for m in ms:
                        nc.vector.scalar_tensor_tensor(
                            g[m], in0=pss[m], scalar=qk,
                            in1=cur[m], op0=ALU.mult, op1=ALU.add)
                    # fp8 tanhs first (they feed the next step's matmuls);
                    # the bf16 double-tanhs for the DVE u-STTs have slack
                    for m in ms:
                        if last:
                            # final tanh in exact bf16 for the u accumulator
                            tau = tau_pool.tile([128, R], BF16,
                                                name=f"tau9_{m}", tag="tau")
                            nc.scalar.activation(tau, g[m], AF.Tanh, scale=ak1)
                            taus[m] = tau
                        else:
                            # tanh straight to fp8 (no separate cast op)
                            dst = t8p[wbuf][m // 2][:, m % 2, :]
                            nc.scalar.activation(dst, g[m], AF.Tanh, scale=ak1)
                            taus[m] = dst
                    if not last:
                        # second, exact bf16 tanhs for the DVE u-STTs (fp8
                        # in1 runs ~2.5x slower on DVE; ACT has slack and u
                        # gets full tanh precision). b3/b4 are deferred
                        # behind f6/f7 so the last matmul-gating fp8 tanhs
                        # land ~2us earlier in ACT's queue; their consumers
                        # (u3/u4) run at the end of the DVE queue.
                        bms = [m for m in ms if m < 3]
                        if q == len(phases) - 1:
                            bms.extend([3, 4])
                        for m in bms:
                            tau = tau_pool.tile([128, R], BF16,
                                                name=f"tau{s + 1}_{m}",
                                                tag="tau")
                            nc.scalar.activation(tau, g[m], AF.Tanh,
                                                 scale=ak1)
                            taus[m] = tau
                    # u-updates for the PREVIOUS phase slot into DVE bubbles
                    # (u3 deferred to the end alongside its late tanh)
                    if prev_ms is not None:
                        for m in prev_ms:
                            if last or m != 3:
                                u_upd(s, m, taus[m], last)
                    prev_ms = ms
                for m in prev_ms:
                    u_upd(s, m, taus[m], last)
                if not last:
                    u_upd(s, 3, taus[3], last)

            # ------------ tail: LN stats + readout (matmul part) ------------
            ones_sb = tail.tile([128, 1], BF16)
            nc.vector.memset(ones_sb, 1.0)
            eps_sb = tail.tile([128, 1], F32)
            nc.vector.memset(eps_sb, EPS)

            s2_sb = tail.tile([1, R], F32)
            y_sb = tail.tile([11, R], F32)

            # y matmuls first (paced by u finalization: the last Euler step
            # processes pairs in reverse, so sweep k in that completion
            # order), s2 after (paced by the sq tiles).
            KORD = [6, 7, 0, 1, 4, 5, 2, 3]
            yps = [psum.tile([11, 512], F32, name=f"yp{n}", tag="mm")
                   for n in range(NS)]
            for ki, k in enumerate(KORD):
                for n in range(NS):
                    sl = slice(n * 512, (n + 1) * 512)
                    nc.tensor.matmul(yps[n], lhsT=w2a_sb[:, k, :],
                                     rhs=u[k][:, sl],
                                     start=(ki == 0), stop=False)
            s2s = [psum.tile([1, 512], F32, name=f"s2p{n}", tag="mm")
                   for n in range(NS)]
            for ki, k in enumerate(KORD):
                for n in range(NS):
                    sl = slice(n * 512, (n + 1) * 512)
                    nc.tensor.matmul(s2s[n], lhsT=ones_sb,
                                     rhs=sq_tiles[k][:, sl],
                                     start=(ki == 0), stop=(ki == KD - 1))
            for n in range(NS):
                nc.scalar.copy(s2_sb[:, n * 512:(n + 1) * 512], s2s[n])
            for ki, k in enumerate(KORD):
                for n in range(NS):
                    sl = slice(n * 512, (n + 1) * 512)
                    nc.tensor.matmul(yps[n], lhsT=w2r_sb[:, k, :],
                                     rhs=u[k][:, sl],
                                     start=False, stop=(ki == KD - 1))
            for n in range(NS):
                nc.scalar.copy(y_sb[:, n * 512:(n + 1) * 512], yps[n])

            mmctx.close()

            def bc(ap, n, axis):
                # broadcast an AP along a new stride-0 dim inserted at `axis`
                newap = list(ap.ap)
                newap.insert(axis, [0, n])
                return bass.AP(tensor=ap.tensor, offset=ap.offset, ap=newap)

            # batched LN + readout: transpose all 8 row-tiles into one stacked
            # [128, rt, 12] psum tile, then do the whole LN/readout chain as
            # [128,8]-wide ops instead of 8 serial per-rt chains.
            tp2ctx = ExitStack()
            tp2 = tp2ctx.enter_context(
                tc.tile_pool(name="tp2", bufs=1, space="PSUM"))
            tp_all = tp2.tile([128, 8, 12], F32, name="tp_all")
            for rt in range(8):
                sl = slice(rt * 128, (rt + 1) * 128)
                nc.tensor.transpose(tp_all[:, rt, 0:11], y_sb[:, sl],
                                    ident[:11, :11])
                nc.tensor.transpose(tp_all[:, rt, 11:12], s2_sb[:, sl],
                                    ident[:1, :1])
            st_all = tail.tile([128, 8, 12], F32, name="st_all")
            nc.vector.tensor_copy(st_all, tp_all)
            yn_all = st_all[:, :, 0:10]
            mu_n = tail.tile([128, 8, 1], F32, name="mu_all")
            nc.scalar.mul(mu_n, st_all[:, :, 10:11], -DT_STEP / D)  # -mean(h)
            ex2 = tail.tile([128, 8, 1], F32, name="ex2_all")
            nc.scalar.mul(ex2, st_all[:, :, 11:12], DT_STEP * DT_STEP / D)
            var = tail.tile([128, 8, 1], F32, name="var_all")
            nc.vector.scalar_tensor_tensor(var, in0=mu_n, scalar=-1.0,
                                           op0=ALU.mult, in1=mu_n,
                                           op1=ALU.mult)      # -mean^2
            nc.vector.tensor_add(var, var, ex2)
            sd = tail.tile([128, 8, 1], F32, name="sd_all")
            nc.scalar.activation(sd, var, AF.Sqrt, bias=eps_sb, scale=1.0)
            inv = tail.tile([128, 8, 1], F32, name="inv_all")
            nc.vector.reciprocal(inv, sd)
            qn = tail.tile([128, 8, 1], F32, name="qn_all")
            nc.vector.tensor_mul(qn, mu_n, inv)                     # -mu*inv

            o_all = tail.tile([128, 8, 10], F32, name="o_all")
            t2_all = tail.tile([128, 8, 10], F32, name="t2_all")
            # o = yn*inv + w1*qn + b2   (stride-0 broadcasts)
            nc.vector.tensor_tensor(o_all, yn_all,
                                    bc(inv[:, :, 0], 10, 2), op=ALU.mult)
            nc.vector.tensor_tensor(t2_all, bc(w1_bc, 8, 1),
                                    bc(qn[:, :, 0], 10, 2), op=ALU.mult)
            nc.vector.tensor_add(o_all, o_all, t2_all)
            nc.vector.tensor_add(o_all, o_all, bc(b2_bc, 8, 1))
            nc.sync.dma_start(out=out.rearrange("(t p) o -> p t o", p=128),
                              in_=o_all)
            tp2ctx.close()

    nc.compile()
    return nc


_NC_CACHE = None


def _get_program():
    global _NC_CACHE
    if _NC_CACHE is None:
        _NC_CACHE = _build_program()
    return _NC_CACHE


def _prepare_in_maps(inputs):
    x = np.asarray(inputs["x"], dtype=np.float32)
    w_enc = np.asarray(inputs["W_enc"], dtype=np.float32)
    w_res = np.asarray(inputs["W_res"], dtype=np.float32)
    w_in = np.asarray(inputs["W_in"], dtype=np.float32)
    bias = np.asarray(inputs["bias"], dtype=np.float32)
    ln_g = np.asarray(inputs["ln_g"], dtype=np.float32)
    ln_b = np.asarray(inputs["ln_b"], dtype=np.float32)
    w_out = np.asarray(inputs["W_out"], dtype=np.float32)
    b_out = np.asarray(inputs["b_out"], dtype=np.float32)

    w_c = (w_enc.T.astype(np.float64) @ w_in.astype(np.float64))
    w2 = w_out * ln_g[None, :]                       # [10, D]

    # encoder weights: 16*W_c in bf16, padded to 896 k-rows, layout [p, k, m]
    wcp = np.zeros((KE * 128, D), np.float64)
    wcp[:KX] = SW * w_c
    wc16 = np.ascontiguousarray(
        wcp.astype(ml_dtypes.bfloat16).reshape(KE, 128, D).transpose(1, 0, 2))

    bias16 = np.ascontiguousarray((SW * bias).reshape(KD, 128).T.astype(np.float32))

    # fp8 recurrent weights, upscaled by SW, layout [p, ksub, m]
    w8 = (SW * w_res).astype(ml_dtypes.float8_e4m3)
    w8 = np.ascontiguousarray(w8.reshape(KD, 128, D).transpose(1, 0, 2))

    # readout: [0.2*W2.T | ones] in bf16 hi + bf16 residual, layout [p, k, o]
    a = np.empty((D, 11), np.float64)
    a[:, :10] = DT_STEP * w2.T.astype(np.float64)
    a[:, 10] = 1.0
    a16 = a.astype(ml_dtypes.bfloat16)
    ar16 = (a - a16.astype(np.float64)).astype(ml_dtypes.bfloat16)
    a16 = np.ascontiguousarray(a16.reshape(KD, 128, 11).transpose(1, 0, 2))
    ar16 = np.ascontiguousarray(ar16.reshape(KD, 128, 11).transpose(1, 0, 2))

    w1v = w2.sum(axis=1).astype(np.float32)
    b2v = (w_out.astype(np.float64) @ ln_b.astype(np.float64)
           + b_out.astype(np.float64)).astype(np.float32)

    shared = {
        "wc": wc16,
        "w8": w8,
        "bias": bias16,
        "w2a": a16,
        "w2r": ar16,
        "w1": np.ascontiguousarray(w1v),
        "b2": np.ascontiguousarray(b2v),
    }
    # x pretransposed + bf16 on host (input marshalling), layout [p, k, b]
    xp = np.zeros((KE * 128, B), ml_dtypes.bfloat16)
    xp[:KX] = x.T.astype(ml_dtypes.bfloat16)
    xp = xp.reshape(KE, 128, B)
    in_maps = []
    for c in range(N_CORES):
        m = dict(shared)
        m["xt"] = np.ascontiguousarray(
            xp[:, :, c * R:(c + 1) * R].transpose(1, 0, 2))
        in_maps.append(m)
    return in_maps


def run(inputs, trace=False, tmpdir=None):
    """Run on 8 NeuronCores; returns (out [8192,10], BassKernelResults)."""
    nc = _get_program()
    in_maps = _prepare_in_maps(inputs)
    res = bass_utils.run_bass_kernel_spmd(
        nc, in_maps, core_ids=list(range(N_CORES)), trace=trace, tmpdir=tmpdir)
    outs = [np.asarray(r["out"]) for r in res.results]
    return np.concatenate(outs, axis=0), res


def kernel(**inputs):
    out, _ = run(inputs, trace=False)
    return out
